# revision 29
# baseline (speedup 1.0000x reference)
"""CourierEncoder fused kernel for 8 Trainium2 NeuronCores.

Data-parallel over the batch: each core processes B/8 = 32768 rows.
Software-pipelined 512-row tiles (all matmuls bf16 -> fp32 PSUM):
  stage A(N):  3 concurrent row-tiled K=2 embed matmuls (biases folded via
               ones rows), Sin on ACT (x|y), LeakyReLU for t on DVE,
               6 L1 matmuls, L1 bias-add on DVE + leaky on GPSIMD
  stage B(N-1): b2 bias matmul (row group 96, concurrent with embeds),
               8 batch-major L2 matmuls (lhsT = h1T slices),
               LeakyReLU on ACT, bf16 DMA store
Stage B is delayed one tile so the PE never stalls on activations.
Inputs are pre-transposed/cast to bf16 on the host ([6, R]: x,1,y,1,t,1)
and loaded to SBUF once -- no per-tile input DMA. Output is stored bf16
and upcast to f32 on the host.
"""

import math

import numpy as np
import ml_dtypes

import concourse.bass as bass
import concourse.tile as tile
import concourse.mybir as mybir
from concourse import bacc
from concourse.bass_utils import run_bass_kernel_spmd

B = 262144
NCORES = 8
R = B // NCORES          # rows per core
TILE = 512               # rows per tile
NT = R // TILE           # tiles per core
PED = 256
NED = 128
CED = 256
Q = PED // 4             # 64
ALPHA = 0.01

F32 = mybir.dt.float32
BF16 = mybir.dt.bfloat16
AF = mybir.ActivationFunctionType
ALU = mybir.AluOpType

_CACHE = {}


def _build():
    nc = bacc.Bacc()
    xyt6 = nc.dram_tensor("xyt6", [6, R], BF16, kind="ExternalInput")
    emb_w66 = nc.dram_tensor("emb_w66", [66, 128], BF16, kind="ExternalInput")
    w1p = nc.dram_tensor("w1p", [128, 3, 2, 128], BF16, kind="ExternalInput")
    w2p = nc.dram_tensor("w2p", [128, 2, 256], BF16, kind="ExternalInput")
    b2rep = nc.dram_tensor("b2rep", [1, 1024], BF16, kind="ExternalInput")
    b1cols = nc.dram_tensor("b1cols", [128, 2], F32, kind="ExternalInput")
    out = nc.dram_tensor("out", [R, 256], BF16, kind="ExternalOutput")

    with tile.TileContext(nc) as tc:
        with (
            tc.tile_pool(name="const", bufs=1) as const,
            tc.tile_pool(name="acts", bufs=2) as acts,
            tc.tile_pool(name="h1", bufs=2) as h1p,
            tc.tile_pool(name="outp", bufs=3) as outp,
            tc.tile_pool(name="ps_emb", bufs=1, space="PSUM") as ps_emb,
            tc.tile_pool(name="ps_l1", bufs=1, space="PSUM") as ps_l1,
            tc.tile_pool(name="ps_l2", bufs=1, space="PSUM") as ps_l2,
        ):
            # -- constants ---------------------------------------------------
            emb_w_sb = const.tile([66, 128], BF16)
            w1_sb = const.tile([128, 3, 2, 128], BF16)
            w2_sb = const.tile([128, 2, 256], BF16)
            ones96 = const.tile([97, 128], BF16)
            b2_96 = const.tile([97, 1024], BF16)
            b1_sb = const.tile([128, 2], F32)
            xyt_all = const.tile([66, R], BF16)
            # warm the ACT table (trig_and_small) concurrently with the DMAs
            warm_sb = const.tile([1, 16], F32)
            nc.vector.memset(warm_sb, 0.0)
            nc.scalar.activation(out=warm_sb, in_=warm_sb, func=AF.Sin)

            # startup DMAs: small heads first (first HEADT tiles of each input
            # row pair) so the pipeline can start while the rest streams in
            HEADT = 16 * TILE
            nc.sync.dma_start(out=emb_w_sb, in_=emb_w66[:, :])
            for c in range(3):
                nc.sync.dma_start(out=xyt_all[32 * c:32 * c + 2, 0:HEADT],
                                  in_=xyt6[2 * c:2 * c + 2, 0:HEADT])
            for c in range(3):
                nc.sync.dma_start(out=xyt_all[32 * c:32 * c + 2, HEADT:R],
                                  in_=xyt6[2 * c:2 * c + 2, HEADT:R])
            nc.scalar.dma_start(out=w1_sb, in_=w1p[:, :, :, :])
            nc.scalar.dma_start(out=w2_sb, in_=w2p[:, :, :])
            nc.gpsimd.dma_start(out=b2_96[96:97, :], in_=b2rep[:, :])
            nc.gpsimd.dma_start(out=b1_sb, in_=b1cols[:, :])
            nc.vector.memset(ones96[96:97, :], 1.0)

            def stage_b(h1T, it):
                """b2 bias + L2 matmuls + LeakyReLU + store for tile `it`."""
                base = it * TILE
                l2_ps = ps_l2.tile([128, 4, 256], F32, tag="l2", name="l2_ps")
                for half in range(2):
                    nc.tensor.matmul(
                        l2_ps[:, 2 * half:2 * half + 2, :],
                        ones96[96:97, :],
                        b2_96[96:97, 512 * half:512 * half + 512],
                        start=True, stop=False,
                        skip_group_check=True,
                        tile_position=(96, 0),
                    )
                for kc in range(2):
                    for r in range(4):
                        nc.tensor.matmul(
                            l2_ps[:, r, :],
                            h1T[:, kc, r * 128:(r + 1) * 128],
                            w2_sb[:, kc, :],
                            start=False, stop=(kc == 1),
                            skip_group_check=True,
                        )
                o_sb = outp.tile([128, 4, 256], BF16, bufs=4, name="o_sb")
                nc.scalar.activation(out=o_sb, in_=l2_ps,
                                     func=AF.Prelu, alpha=ALPHA)
                nc.sync.dma_start(
                    out=out[base:base + TILE, :].rearrange("(r p) m -> p r m", p=128),
                    in_=o_sb,
                )

            def stage_a1(it):
                """Embeddings + Sin/t-act for tile `it` -> hT."""
                base = it * TILE
                emb_xy = ps_emb.tile([128, 2, TILE], F32, tag="xy", name="emb_xy")
                emb_t = ps_emb.tile([128, TILE], F32, tag="t", bufs=2,
                                    name="emb_t")
                for c in range(2):
                    nc.tensor.matmul(
                        emb_xy[:, c, :],
                        emb_w_sb[32 * c:32 * c + 2, :],
                        xyt_all[32 * c:32 * c + 2, base:base + TILE],
                        start=True, stop=True,
                        tile_position=(32 * c, 0),
                    )
                nc.tensor.matmul(
                    emb_t,
                    emb_w_sb[64:66, :],
                    xyt_all[64:66, base:base + TILE],
                    start=True, stop=True,
                    tile_position=(64, 0),
                )
                hT = acts.tile([128, 3, TILE], BF16, bufs=3, name="hT")
                nc.scalar.activation(out=hT[:, 0:2, :], in_=emb_xy,
                                     func=AF.Sin)
                nc.scalar.activation(out=hT[:, 2, :], in_=emb_t,
                                     func=AF.Prelu, alpha=ALPHA)
                return hT

            def stage_a2(hT, it):
                """L1 matmuls + DVE tails for tile `it` -> h1T."""
                l1_ps = [
                    ps_l1.tile([128, TILE], F32, tag="l1a", name="l1a_ps"),
                    ps_l1.tile([128, TILE], F32, tag="l1b", name="l1b_ps"),
                ]
                for mc in range(2):
                    for kc in range(3):
                        nc.tensor.matmul(
                            l1_ps[mc],
                            w1_sb[:, kc, mc, :],
                            hT[:, kc, :],
                            start=(kc == 0), stop=(kc == 2),
                        )
                u_sb = acts.tile([128, 2, TILE], BF16, tag="u", bufs=3, name="u_sb")
                h1T = h1p.tile([128, 2, TILE], BF16, bufs=3, name="h1T")
                for mc in range(2):
                    nc.vector.tensor_scalar_add(out=u_sb[:, mc, :],
                                                in0=l1_ps[mc],
                                                scalar1=b1_sb[:, mc:mc + 1])
                    nc.vector.scalar_tensor_tensor(
                        out=h1T[:, mc, :], in0=u_sb[:, mc, :], scalar=ALPHA,
                        in1=u_sb[:, mc, :], op0=ALU.mult, op1=ALU.max,
                    )
                return h1T

            # three-deep software pipeline: embeds/sin run two tiles ahead
            # (L1 then has no activation dependency and leads the PE stream),
            # the L2 stage runs one tile behind
            hts = {0: stage_a1(0), 1: stage_a1(1)}
            prev = None
            for it in range(NT):
                h1T = stage_a2(hts.pop(it), it)
                if it + 2 < NT:
                    hts[it + 2] = stage_a1(it + 2)
                if prev is not None:
                    stage_b(*prev)
                prev = (h1T, it)

            stage_b(*prev)
    nc.finalize()
    return nc


def _prep_weights(inputs):
    f = {k: np.asarray(v, dtype=np.float32) for k, v in inputs.items()}
    bf = ml_dtypes.bfloat16

    emb_w66 = np.zeros((66, 128), dtype=np.float32)
    emb_w66[0] = np.concatenate([f["w_sx"].ravel(), f["w_cx"].ravel()])
    emb_w66[1] = np.concatenate([f["b_sx"], f["b_cx"] + math.pi / 2])
    emb_w66[32] = np.concatenate([f["w_sy"].ravel(), f["w_cy"].ravel()])
    emb_w66[33] = np.concatenate([f["b_sy"], f["b_cy"] + math.pi / 2])
    emb_w66[64] = f["w_t"].ravel()
    emb_w66[65] = f["b_t"]
    emb_w66 = emb_w66.astype(bf)

    b1cols = np.stack([f["b1"][0:128], f["b1"][128:256]], axis=1)
    b1cols = np.ascontiguousarray(b1cols, dtype=np.float32)

    w1p = f["w1"].reshape(3, 128, 2, 128).transpose(1, 0, 2, 3).astype(bf)
    w2p = f["w2"].reshape(2, 128, 256).transpose(1, 0, 2).astype(bf)

    b2rep = np.tile(f["b2"], 4)[None, :].astype(bf)

    return {
        "emb_w66": emb_w66,
        "b1cols": b1cols,
        "w1p": np.ascontiguousarray(w1p),
        "w2p": np.ascontiguousarray(w2p),
        "b2rep": b2rep,
    }


def kernel(**inputs):
    if "nc" not in _CACHE:
        _CACHE["nc"] = _build()
    nc = _CACHE["nc"]

    w = _prep_weights(inputs)
    bf = ml_dtypes.bfloat16
    xy = np.asarray(inputs["xy"], dtype=np.float32)
    t = np.asarray(inputs["t"], dtype=np.float32)
    xyt6 = np.ones((6, B), dtype=bf)
    xyt6[0] = xy[:, 0].astype(bf)
    xyt6[2] = xy[:, 1].astype(bf)
    xyt6[4] = t[:, 0].astype(bf)

    in_maps = []
    for c in range(NCORES):
        lo, hi = c * R, (c + 1) * R
        in_maps.append({
            "xyt6": np.ascontiguousarray(xyt6[:, lo:hi]), **w,
        })

    res = run_bass_kernel_spmd(nc, in_maps, core_ids=list(range(NCORES)))
    _CACHE["last_res"] = res
    out = np.concatenate([res.results[c]["out"] for c in range(NCORES)], axis=0)
    return out.astype(np.float32)


# revision 30
# speedup vs baseline: 1.0379x; 1.0379x over previous
"""CourierEncoder fused kernel for 8 Trainium2 NeuronCores.

Data-parallel over the batch: each core processes B/8 = 32768 rows.
Software-pipelined 512-row tiles (all matmuls bf16 -> fp32 PSUM):
  stage A(N):  3 concurrent row-tiled K=2 embed matmuls (biases folded via
               ones rows), Sin on ACT (x|y), LeakyReLU for t on DVE,
               6 L1 matmuls, L1 bias-add on DVE + leaky on GPSIMD
  stage B(N-1): b2 bias matmul (row group 96, concurrent with embeds),
               8 batch-major L2 matmuls (lhsT = h1T slices),
               LeakyReLU on ACT, bf16 DMA store
Stage B is delayed one tile so the PE never stalls on activations.
Inputs are pre-transposed/cast to bf16 on the host ([6, R]: x,1,y,1,t,1)
and loaded to SBUF once -- no per-tile input DMA. Output is stored bf16
and upcast to f32 on the host.
"""

import math

import numpy as np
import ml_dtypes

import concourse.bass as bass
import concourse.tile as tile
import concourse.mybir as mybir
from concourse import bacc
from concourse.bass_utils import run_bass_kernel_spmd

B = 262144
NCORES = 8
R = B // NCORES          # rows per core
TILE = 512               # rows per tile
NT = R // TILE           # tiles per core
PED = 256
NED = 128
CED = 256
Q = PED // 4             # 64
ALPHA = 0.01

F32 = mybir.dt.float32
BF16 = mybir.dt.bfloat16
AF = mybir.ActivationFunctionType
ALU = mybir.AluOpType

_CACHE = {}


def _build():
    nc = bacc.Bacc()
    xyt6 = nc.dram_tensor("xyt6", [6, R], BF16, kind="ExternalInput")
    emb_w66 = nc.dram_tensor("emb_w66", [66, 128], BF16, kind="ExternalInput")
    w1p = nc.dram_tensor("w1p", [128, 3, 2, 128], BF16, kind="ExternalInput")
    w2p = nc.dram_tensor("w2p", [128, 2, 256], BF16, kind="ExternalInput")
    b2rep = nc.dram_tensor("b2rep", [1, 1024], BF16, kind="ExternalInput")
    b1cols = nc.dram_tensor("b1cols", [128, 2], F32, kind="ExternalInput")
    out = nc.dram_tensor("out", [R, 256], BF16, kind="ExternalOutput")

    with tile.TileContext(nc) as tc:
        with (
            tc.tile_pool(name="const", bufs=1) as const,
            tc.tile_pool(name="acts", bufs=2) as acts,
            tc.tile_pool(name="h1", bufs=2) as h1p,
            tc.tile_pool(name="outp", bufs=3) as outp,
            tc.tile_pool(name="ps_emb", bufs=1, space="PSUM") as ps_emb,
            tc.tile_pool(name="ps_l1", bufs=1, space="PSUM") as ps_l1,
            tc.tile_pool(name="ps_l2", bufs=1, space="PSUM") as ps_l2,
        ):
            # -- constants ---------------------------------------------------
            emb_w_sb = const.tile([66, 128], BF16)
            w1_sb = const.tile([128, 3, 2, 128], BF16)
            w2_sb = const.tile([128, 2, 256], BF16)
            ones96 = const.tile([97, 128], BF16)
            b2_96 = const.tile([97, 1024], BF16)
            b1_sb = const.tile([128, 2], F32)
            xyt_all = const.tile([66, R], BF16)
            # warm the ACT table (trig_and_small) concurrently with the DMAs
            warm_sb = const.tile([1, 16], F32)
            nc.vector.memset(warm_sb, 0.0)
            nc.scalar.activation(out=warm_sb, in_=warm_sb, func=AF.Sin)

            # startup DMAs: small heads first (first HEADT tiles of each input
            # row pair) so the pipeline can start while the rest streams in
            HEADT = 4 * TILE
            nc.sync.dma_start(out=emb_w_sb, in_=emb_w66[:, :])
            for c in range(3):
                nc.sync.dma_start(out=xyt_all[32 * c:32 * c + 2, 0:HEADT],
                                  in_=xyt6[2 * c:2 * c + 2, 0:HEADT])
            for c in range(3):
                nc.sync.dma_start(out=xyt_all[32 * c:32 * c + 2, HEADT:R],
                                  in_=xyt6[2 * c:2 * c + 2, HEADT:R])
            nc.scalar.dma_start(out=w1_sb, in_=w1p[:, :, :, :])
            nc.scalar.dma_start(out=w2_sb, in_=w2p[:, :, :])
            nc.gpsimd.dma_start(out=b2_96[96:97, :], in_=b2rep[:, :])
            nc.gpsimd.dma_start(out=b1_sb, in_=b1cols[:, :])
            nc.vector.memset(ones96[96:97, :], 1.0)

            def stage_b(h1T, it):
                """b2 bias + L2 matmuls + LeakyReLU + store for tile `it`."""
                base = it * TILE
                l2_ps = ps_l2.tile([128, 4, 256], F32, tag="l2", name="l2_ps")
                for half in range(2):
                    nc.tensor.matmul(
                        l2_ps[:, 2 * half:2 * half + 2, :],
                        ones96[96:97, :],
                        b2_96[96:97, 512 * half:512 * half + 512],
                        start=True, stop=False,
                        skip_group_check=True,
                        tile_position=(96, 0),
                    )
                for kc in range(2):
                    for r in range(4):
                        nc.tensor.matmul(
                            l2_ps[:, r, :],
                            h1T[:, kc, r * 128:(r + 1) * 128],
                            w2_sb[:, kc, :],
                            start=False, stop=(kc == 1),
                            skip_group_check=True,
                        )
                o_sb = outp.tile([128, 4, 256], BF16, name="o_sb")
                nc.scalar.activation(out=o_sb, in_=l2_ps,
                                     func=AF.Prelu, alpha=ALPHA)
                nc.sync.dma_start(
                    out=out[base:base + TILE, :].rearrange("(r p) m -> p r m", p=128),
                    in_=o_sb,
                )

            def stage_a1(it):
                """Embeddings + Sin/t-act for tile `it` -> hT."""
                base = it * TILE
                emb_xy = ps_emb.tile([128, 2, TILE], F32, tag="xy", name="emb_xy")
                emb_t = ps_emb.tile([128, TILE], F32, tag="t", bufs=2,
                                    name="emb_t")
                for c in range(2):
                    nc.tensor.matmul(
                        emb_xy[:, c, :],
                        emb_w_sb[32 * c:32 * c + 2, :],
                        xyt_all[32 * c:32 * c + 2, base:base + TILE],
                        start=True, stop=True,
                        tile_position=(32 * c, 0),
                    )
                nc.tensor.matmul(
                    emb_t,
                    emb_w_sb[64:66, :],
                    xyt_all[64:66, base:base + TILE],
                    start=True, stop=True,
                    tile_position=(64, 0),
                )
                hT = acts.tile([128, 3, TILE], BF16, bufs=3, name="hT")
                nc.scalar.activation(out=hT[:, 0:2, :], in_=emb_xy,
                                     func=AF.Sin)
                nc.scalar.activation(out=hT[:, 2, :], in_=emb_t,
                                     func=AF.Prelu, alpha=ALPHA)
                return hT

            def stage_a2(hT, it):
                """L1 matmuls + DVE tails for tile `it` -> h1T."""
                l1_ps = [
                    ps_l1.tile([128, TILE], F32, tag="l1a", name="l1a_ps"),
                    ps_l1.tile([128, TILE], F32, tag="l1b", name="l1b_ps"),
                ]
                for mc in range(2):
                    for kc in range(3):
                        nc.tensor.matmul(
                            l1_ps[mc],
                            w1_sb[:, kc, mc, :],
                            hT[:, kc, :],
                            start=(kc == 0), stop=(kc == 2),
                        )
                u_sb = acts.tile([128, 2, TILE], BF16, tag="u", name="u_sb")
                h1T = h1p.tile([128, 2, TILE], BF16, name="h1T")
                for mc in range(2):
                    nc.vector.tensor_scalar_add(out=u_sb[:, mc, :],
                                                in0=l1_ps[mc],
                                                scalar1=b1_sb[:, mc:mc + 1])
                    nc.vector.scalar_tensor_tensor(
                        out=h1T[:, mc, :], in0=u_sb[:, mc, :], scalar=ALPHA,
                        in1=u_sb[:, mc, :], op0=ALU.mult, op1=ALU.max,
                    )
                return h1T

            # three-deep software pipeline: embeds/sin run two tiles ahead
            # (L1 then has no activation dependency and leads the PE stream),
            # the L2 stage runs one tile behind
            hts = {0: stage_a1(0), 1: stage_a1(1)}
            prev = None
            for it in range(NT):
                h1T = stage_a2(hts.pop(it), it)
                if it + 2 < NT:
                    hts[it + 2] = stage_a1(it + 2)
                if prev is not None:
                    stage_b(*prev)
                prev = (h1T, it)

            stage_b(*prev)
    nc.finalize()
    return nc


def _prep_weights(inputs):
    f = {k: np.asarray(v, dtype=np.float32) for k, v in inputs.items()}
    bf = ml_dtypes.bfloat16

    emb_w66 = np.zeros((66, 128), dtype=np.float32)
    emb_w66[0] = np.concatenate([f["w_sx"].ravel(), f["w_cx"].ravel()])
    emb_w66[1] = np.concatenate([f["b_sx"], f["b_cx"] + math.pi / 2])
    emb_w66[32] = np.concatenate([f["w_sy"].ravel(), f["w_cy"].ravel()])
    emb_w66[33] = np.concatenate([f["b_sy"], f["b_cy"] + math.pi / 2])
    emb_w66[64] = f["w_t"].ravel()
    emb_w66[65] = f["b_t"]
    emb_w66 = emb_w66.astype(bf)

    b1cols = np.stack([f["b1"][0:128], f["b1"][128:256]], axis=1)
    b1cols = np.ascontiguousarray(b1cols, dtype=np.float32)

    w1p = f["w1"].reshape(3, 128, 2, 128).transpose(1, 0, 2, 3).astype(bf)
    w2p = f["w2"].reshape(2, 128, 256).transpose(1, 0, 2).astype(bf)

    b2rep = np.tile(f["b2"], 4)[None, :].astype(bf)

    return {
        "emb_w66": emb_w66,
        "b1cols": b1cols,
        "w1p": np.ascontiguousarray(w1p),
        "w2p": np.ascontiguousarray(w2p),
        "b2rep": b2rep,
    }


def kernel(**inputs):
    if "nc" not in _CACHE:
        _CACHE["nc"] = _build()
    nc = _CACHE["nc"]

    w = _prep_weights(inputs)
    bf = ml_dtypes.bfloat16
    xy = np.asarray(inputs["xy"], dtype=np.float32)
    t = np.asarray(inputs["t"], dtype=np.float32)
    xyt6 = np.ones((6, B), dtype=bf)
    xyt6[0] = xy[:, 0].astype(bf)
    xyt6[2] = xy[:, 1].astype(bf)
    xyt6[4] = t[:, 0].astype(bf)

    in_maps = []
    for c in range(NCORES):
        lo, hi = c * R, (c + 1) * R
        in_maps.append({
            "xyt6": np.ascontiguousarray(xyt6[:, lo:hi]), **w,
        })

    res = run_bass_kernel_spmd(nc, in_maps, core_ids=list(range(NCORES)))
    _CACHE["last_res"] = res
    out = np.concatenate([res.results[c]["out"] for c in range(NCORES)], axis=0)
    return out.astype(np.float32)
